# revision 2
# baseline (speedup 1.0000x reference)
"""CTRNN forward kernel for Trainium2 (8 NeuronCores, data-parallel over batch).

Reference computation (per step t, dt=0.02):
    h = h*(1-dt) + dt*(tanh(h) @ J.T + v_t @ Bmat.T)
    out_t = tanh(h) @ W_ro.T

Design (v2, fp16 weight path):
  - Per core: B_LOC=16 batch rows. Everything in "hT" layout: hidden on
    partitions (4 chunks of 128), batch on the free dim.
  - h lives permanently in PSUM, scaled by HSCALE=64 to keep the fp16
    weights (64*dt*J) well clear of the fp16 subnormal range:
      H_t = 0.98*H_{t-1} + (64*dt)*(J y_{t-1} + B v_t),  y = tanh(H/64)
    The /64 rides ACT's free input-scale; readout uses unscaled y.
  - Four PSUM banks, one per 128-row output block b. Per step, per block:
      stt  (DVE):  H_b = 0.98*H_b + bv_t[b]         (bv precomputed)
      4 MM (PE) :  H_b += (64 dt J)_b,ci @ y_{t-1},ci   fp16, M=128 -> FWL
      ACT (ScE) :  y_t[b] = tanh(H_b / 64) -> fp16 SBUF ring
    stt is emitted before the MMs (J pre-scaled by 64*dt only), so each
    block's DVE/ACT tail hides under the other blocks' matmuls.
  - stt emission order [1,2,3,0] + block-0 MM chunk order [3,0,1,2] make
    the first MM of each step wait on the two newest ticks (covering all
    older ones), so nearly every PE instruction needs <=1 sem wait.
  - y ring buffer holds RO=32 steps; readout is batched: every 32 steps,
    4 accumulating MMs (lhsT = W_ro chunk [128,1], rhs = y block [128,512])
    produce out[1, (t,b)] in PSUM, copied+DMA'd out. Host de-interleaves.
  - bv outer products (64*dt*B x v_t) for the NEXT 128-step block are
    built by K=1 PE matmuls + DVE copies, spread one pair per 8 steps.
"""

import math
import sys

import numpy as np

sys.path.insert(0, "/opt/trn_rl_repo")

DT = 0.02
DECAY = 1.0 - DT          # 0.98
HSCALE = 64.0             # h kept as 64*h in PSUM (fp16 subnormal guard)
HIDDEN = 512
BATCH = 128
T_FULL = 1024
N_CORES = 8
B_LOC = BATCH // N_CORES  # 16
CB = HIDDEN // 128        # 4 hidden chunks / output blocks


def build_nc(T=T_FULL, lbv=128, ro=32):
    import concourse.bass as bass
    import concourse.tile as tile
    from concourse import bacc, mybir

    f32 = mybir.dt.float32
    f16 = mybir.dt.float16
    nc = bacc.Bacc()

    jt_h = nc.declare_dram_parameter("JT", [HIDDEN, HIDDEN], f16, isOutput=False)
    bmr_h = nc.declare_dram_parameter("bmr", [1, HIDDEN], f16, isOutput=False)
    wrt_h = nc.declare_dram_parameter("wrt", [128, CB], f16, isOutput=False)
    velt_h = nc.declare_dram_parameter("velT", [T, B_LOC], f16, isOutput=False)
    out_h = nc.declare_dram_parameter("out", [1, T * B_LOC], f32, isOutput=True)

    nblk = (T + lbv - 1) // lbv
    nro = (T + ro - 1) // ro
    rosz = ro * B_LOC  # 512 = one PSUM bank of fp32

    with tile.TileContext(nc) as tc:
        with (
            tc.tile_pool(name="singles", bufs=1) as singles,
            tc.tile_pool(name="ybp", bufs=2) as ybp,
            tc.tile_pool(name="velp", bufs=2) as velp,
            tc.tile_pool(name="bvpp", bufs=2) as bvpp,
            tc.tile_pool(name="osbp", bufs=2) as osbp,
            tc.tile_pool(name="psum", bufs=1, space="PSUM") as pp,
        ):
            # ---- weights staging ----
            jt = singles.tile([128, CB, HIDDEN], f16, tag="jt")  # (64*dt*J)^T tiles
            nc.sync.dma_start(out=jt, in_=jt_h.rearrange("(c p) i -> p c i", p=128))
            bmr = singles.tile([1, HIDDEN], f16, tag="bmr")  # 64*dt*Bmat row
            nc.sync.dma_start(out=bmr, in_=bmr_h[:, :])
            wrt = singles.tile([128, CB], f16, tag="wrt")
            nc.sync.dma_start(out=wrt, in_=wrt_h[:, :])

            zrow = singles.tile([1, 128], f32, tag="zrow")
            nc.vector.memset(zrow, 0.0)

            # y_{-1} = tanh(0) = 0
            y0 = singles.tile([128, CB, B_LOC], f16, tag="y0")
            nc.vector.memset(
                y0.rearrange("p c b -> p (c b)").bitcast(f32), 0.0
            )

            # h (scaled): one PSUM bank per 128-row output block
            psum_z = [
                pp.tile([128, B_LOC], f32, tag=f"z{b}", name=f"psum_z{b}")
                for b in range(CB)
            ]
            pjunk = pp.tile([1, 8], f32, tag="junk", name="psum_junk")

            def absorb(src_1el):
                if src_1el.dtype != f32:
                    src_1el = src_1el.bitcast(f32)
                nc.tensor.matmul(
                    out=pjunk[0:1, 0:1],
                    lhsT=src_1el,
                    rhs=src_1el,
                    start=True,
                    stop=True,
                    skip_group_check=True,
                )

            # claim + zero the h banks (sets has_written so later MMs accumulate)
            for b in range(CB):
                nc.tensor.matmul(
                    out=psum_z[b],
                    lhsT=zrow[0:1, 0:128],
                    rhs=zrow[0:1, 0:B_LOC],
                    start=True,
                    stop=True,
                    skip_group_check=True,
                )

            # soak up staging DMA/memset ticks one at a time
            absorb(jt[0:1, 0, 0:1])
            absorb(wrt[0:1, 0:1])
            absorb(bmr[0:1, 0:1])

            def build_bv_pair(r, velb_t, bvp_t):
                # pair r: chunk c = r // 4, quarter q = r % 4 of the block
                c, q = divmod(r, 4)
                q0 = q * 512
                t0 = q0 // B_LOC
                pbv = pp.tile([128, 512], f32, tag="pbv", bufs=1, name="psum_bv")
                nc.tensor.matmul(
                    out=pbv,
                    lhsT=bmr[0:1, 128 * c : 128 * (c + 1)],
                    rhs=velb_t[0:1, q0 : q0 + 512],
                    start=True,
                    stop=True,
                    skip_group_check=True,
                )
                nc.vector.tensor_copy(
                    bvp_t[:, t0 : t0 + 512 // B_LOC, c, :],
                    pbv.rearrange("p (t b) -> p t b", b=B_LOC),
                )

            def dma_velb(t0):
                velb_t = velp.tile([1, lbv * B_LOC], f16, tag="velB")
                nc.sync.dma_start(
                    out=velb_t,
                    in_=velt_h[t0 : t0 + lbv, :].rearrange("t b -> (t b)").unsqueeze(0),
                )
                return velb_t

            def emit_readout(k, ytile):
                pro = pp.tile([1, rosz], f32, tag="ro", bufs=1, name="psum_ro")
                for c in range(CB):
                    nc.tensor.matmul(
                        out=pro,
                        lhsT=wrt[:, c : c + 1],
                        rhs=ytile[:, c, :, :].rearrange("p t b -> p (t b)"),
                        start=(c == 0),
                        stop=(c == CB - 1),
                        skip_group_check=True,
                    )
                osb = osbp.tile([1, rosz], f32, tag="osb", name="out_sb")
                nc.vector.tensor_copy(osb, pro)
                nc.sync.dma_start(
                    out=out_h[0:1, k * rosz : (k + 1) * rosz], in_=osb
                )

            # prologue: vel block 0 + its bv outer products
            velb = dma_velb(0)
            bvp_cur = bvpp.tile([128, lbv, CB, B_LOC], f16, tag="bvp")
            for r in range(16):
                build_bv_pair(r, velb, bvp_cur)
            bvp_next = None
            velb_next = None

            yb_cur = None
            yb_prev = None
            for t in range(T):
                blk, j = divmod(t, lbv)
                rob, rj = divmod(t, ro)

                if rj == 0:
                    yb_prev = yb_cur
                    yb_cur = ybp.tile([128, CB, ro, B_LOC], f16, tag="yb")

                if j == 0 and blk + 1 < nblk:
                    velb_next = dma_velb(t + lbv)
                    bvp_next = bvpp.tile([128, lbv, CB, B_LOC], f16, tag="bvp")

                # spread next block's bv build: one pair per 8 steps
                if blk + 1 < nblk and j >= 8 and j % 8 == 0:
                    build_bv_pair(j // 8 - 1, velb_next, bvp_next)
                    if j == 120:
                        build_bv_pair(15, velb_next, bvp_next)

                # batched readout of the previous 32-step block
                if rj == 4 and rob >= 1:
                    emit_readout(rob - 1, yb_prev)

                # ---- the step ----
                # decay + input; block 0 last so its DVE tick covers the rest
                for b in (1, 2, 3, 0):
                    nc.vector.scalar_tensor_tensor(
                        out=psum_z[b],
                        in0=psum_z[b],
                        scalar=float(DECAY),
                        in1=bvp_cur[:, j, b, :],
                        op0=mybir.AluOpType.mult,
                        op1=mybir.AluOpType.add,
                    )
                # recurrent matmuls; first MM (b0,ci3) waits the newest ACT
                # tick of step t-1, covering every other y read this step
                if t == 0:
                    ysl = lambda ci: y0[:, ci, :]
                elif rj == 0:
                    ysl = lambda ci: yb_prev[:, ci, ro - 1, :]
                else:
                    ysl = lambda ci, _s=rj - 1: yb_cur[:, ci, _s, :]
                for b in range(CB):
                    cis = (3, 0, 1, 2) if b == 0 else (0, 1, 2, 3)
                    for ci in cis:
                        nc.tensor.matmul(
                            out=psum_z[b],
                            lhsT=jt[:, ci, 128 * b : 128 * (b + 1)],
                            rhs=ysl(ci),
                            start=False,
                            stop=False,
                            skip_group_check=True,
                        )
                # y_t = tanh(H/64)
                for b in range(CB):
                    nc.scalar.activation(
                        out=yb_cur[:, b, rj, :],
                        in_=psum_z[b],
                        func=mybir.ActivationFunctionType.Tanh,
                        scale=1.0 / HSCALE,
                    )

                if j == lbv - 1 and blk + 1 < nblk:
                    bvp_cur = bvp_next
                    velb = velb_next

            emit_readout(nro - 1, yb_cur)

    nc.compile()
    return nc


_NC_CACHE = {}


def _get_nc(**kw):
    key = tuple(sorted(kw.items()))
    if key not in _NC_CACHE:
        _NC_CACHE[key] = build_nc(**kw)
    return _NC_CACHE[key]


def make_in_maps(vel, J, Bmat, W_ro):
    vel = np.asarray(vel, dtype=np.float32)[:, :, 0]          # [B, T]
    J = np.asarray(J, dtype=np.float32)
    Bmat = np.asarray(Bmat, dtype=np.float32)
    W_ro = np.asarray(W_ro, dtype=np.float32)

    jt = np.ascontiguousarray((HSCALE * DT * J).T).astype(np.float16)
    bmr = np.ascontiguousarray(
        (HSCALE * DT * Bmat[:, 0]).reshape(1, HIDDEN)
    ).astype(np.float16)
    wrt = np.ascontiguousarray(W_ro[0].reshape(CB, 128).T).astype(np.float16)
    return [
        {
            "JT": jt,
            "bmr": bmr,
            "wrt": wrt,
            "velT": np.ascontiguousarray(
                vel[c * B_LOC : (c + 1) * B_LOC].T
            ).astype(np.float16),
        }
        for c in range(N_CORES)
    ]


def kernel(vel, J, Bmat, W_ro, _trace=False, **build_kw):
    from concourse.bass_utils import run_bass_kernel_spmd

    nc = _get_nc(**build_kw)
    in_maps = make_in_maps(vel, J, Bmat, W_ro)
    res = run_bass_kernel_spmd(nc, in_maps, list(range(N_CORES)), trace=_trace)
    # out[0, t*B_LOC + b] = readout(batch row b, step t)
    out = np.stack(
        [r["out"].reshape(T_FULL, B_LOC).T for r in res.results], axis=0
    ).reshape(BATCH, T_FULL)
    out = out[:, :, None].astype(np.float32)
    if _trace:
        kernel.last_results = res
    return out


kernel.last_results = None


# revision 4
# speedup vs baseline: 3.3261x; 3.3261x over previous
"""CTRNN forward kernel for Trainium2 (8 NeuronCores, data-parallel over batch).

Reference computation (per step t, dt=0.02):
    h = h*(1-dt) + dt*(tanh(h) @ J.T + v_t @ Bmat.T)
    out_t = tanh(h) @ W_ro.T

Design (v2, fp16 weight path):
  - Per core: B_LOC=16 batch rows. Everything in "hT" layout: hidden on
    partitions (4 chunks of 128), batch on the free dim.
  - h lives permanently in PSUM, scaled by HSCALE=64 to keep the fp16
    weights (64*dt*J) well clear of the fp16 subnormal range:
      H_t = 0.98*H_{t-1} + (64*dt)*(J y_{t-1} + B v_t),  y = tanh(H/64)
    The /64 rides ACT's free input-scale; readout uses unscaled y.
  - Four PSUM banks, one per 128-row output block b. Per step, per block:
      stt  (DVE):  H_b = 0.98*H_b + bv_t[b]         (bv precomputed)
      4 MM (PE) :  H_b += (64 dt J)_b,ci @ y_{t-1},ci   fp16, M=128 -> FWL
      ACT (ScE) :  y_t[b] = tanh(H_b / 64) -> fp16 SBUF ring
    stt is emitted before the MMs (J pre-scaled by 64*dt only), so each
    block's DVE/ACT tail hides under the other blocks' matmuls.
  - stt emission order [1,2,3,0] + block-0 MM chunk order [3,0,1,2] make
    the first MM of each step wait on the two newest ticks (covering all
    older ones), so nearly every PE instruction needs <=1 sem wait.
  - y ring buffer holds RO=32 steps; readout is batched: every 32 steps,
    4 accumulating MMs (lhsT = W_ro chunk [128,1], rhs = y block [128,512])
    produce out[1, (t,b)] in PSUM, copied+DMA'd out. Host de-interleaves.
  - bv outer products (64*dt*B x v_t) for the NEXT 128-step block are
    built by K=1 PE matmuls + DVE copies, spread one pair per 8 steps.
"""

import math
import sys

import numpy as np

sys.path.insert(0, "/opt/trn_rl_repo")

DT = 0.02
DECAY = 1.0 - DT          # 0.98
HSCALE = 64.0             # h kept as 64*h in PSUM (fp16 subnormal guard)
HIDDEN = 512
BATCH = 128
T_FULL = 1024
N_CORES = 8
B_LOC = BATCH // N_CORES  # 16
CB = HIDDEN // 128        # 4 hidden chunks / output blocks


def build_nc(T=T_FULL, lbv=128, ro=32):
    import concourse.bass as bass
    import concourse.tile as tile
    from concourse import bacc, mybir

    f32 = mybir.dt.float32
    f16 = mybir.dt.float16
    nc = bacc.Bacc()

    jt_h = nc.declare_dram_parameter("JT", [HIDDEN, HIDDEN], f16, isOutput=False)
    bmr_h = nc.declare_dram_parameter("bmr", [1, HIDDEN], f16, isOutput=False)
    wrt_h = nc.declare_dram_parameter("wrt", [128, CB], f16, isOutput=False)
    velt_h = nc.declare_dram_parameter("velT", [T, B_LOC], f16, isOutput=False)
    out_h = nc.declare_dram_parameter("out", [1, T * B_LOC], f32, isOutput=True)

    nblk = (T + lbv - 1) // lbv
    nro = (T + ro - 1) // ro
    rosz = ro * B_LOC  # 512 = one PSUM bank of fp32

    with tile.TileContext(nc) as tc:
        with (
            tc.tile_pool(name="singles", bufs=1) as singles,
            tc.tile_pool(name="ybp", bufs=2) as ybp,
            tc.tile_pool(name="velp", bufs=2) as velp,
            tc.tile_pool(name="bvpp", bufs=2) as bvpp,
            tc.tile_pool(name="osbp", bufs=2) as osbp,
            tc.tile_pool(name="psum", bufs=1, space="PSUM") as pp,
        ):
            # ---- weights staging ----
            jt = singles.tile([128, CB, HIDDEN], f16, tag="jt")  # (64*dt*J)^T tiles
            nc.sync.dma_start(out=jt, in_=jt_h.rearrange("(c p) i -> p c i", p=128))
            bmr = singles.tile([1, HIDDEN], f16, tag="bmr")  # 64*dt*Bmat row
            nc.sync.dma_start(out=bmr, in_=bmr_h[:, :])
            wrt = singles.tile([128, CB], f16, tag="wrt")
            nc.sync.dma_start(out=wrt, in_=wrt_h[:, :])

            zrow = singles.tile([1, 128], f32, tag="zrow")
            nc.vector.memset(zrow, 0.0)

            # y_{-1} = tanh(0) = 0
            y0 = singles.tile([128, CB, B_LOC], f16, tag="y0")
            nc.vector.memset(
                y0.rearrange("p c b -> p (c b)").bitcast(f32), 0.0
            )

            # h (scaled): one PSUM bank per 128-row output block
            psum_z = [
                pp.tile([128, B_LOC], f32, tag=f"z{b}", name=f"psum_z{b}")
                for b in range(CB)
            ]
            pjunk = pp.tile([1, 8], f32, tag="junk", name="psum_junk")

            def absorb(src_1el):
                if src_1el.dtype != f32:
                    # 16-bit sources: pass a 2-element slice -> one f32
                    src_1el = src_1el.bitcast(f32)
                nc.tensor.matmul(
                    out=pjunk[0:1, 0:1],
                    lhsT=src_1el,
                    rhs=src_1el,
                    start=True,
                    stop=True,
                    skip_group_check=True,
                )

            # claim + zero the h banks (sets has_written so later MMs accumulate)
            for b in range(CB):
                nc.tensor.matmul(
                    out=psum_z[b],
                    lhsT=zrow[0:1, 0:128],
                    rhs=zrow[0:1, 0:B_LOC],
                    start=True,
                    stop=True,
                    skip_group_check=True,
                )

            # soak up staging DMA/memset ticks one at a time
            absorb(jt[0:1, 0, 0:2])
            absorb(wrt[0:1, 0:2])
            absorb(bmr[0:1, 0:2])

            def build_bv_pair(r, velb_t, bvp_t):
                # pair r: chunk c = r // 4, quarter q = r % 4 of the block
                c, q = divmod(r, 4)
                q0 = q * 512
                t0 = q0 // B_LOC
                pbv = pp.tile([128, 512], f32, tag="pbv", bufs=1, name="psum_bv")
                nc.tensor.matmul(
                    out=pbv,
                    lhsT=bmr[0:1, 128 * c : 128 * (c + 1)],
                    rhs=velb_t[0:1, q0 : q0 + 512],
                    start=True,
                    stop=True,
                    skip_group_check=True,
                )
                nc.vector.tensor_copy(
                    bvp_t[:, t0 : t0 + 512 // B_LOC, c, :],
                    pbv.rearrange("p (t b) -> p t b", b=B_LOC),
                )

            def dma_velb(t0):
                velb_t = velp.tile([1, lbv * B_LOC], f16, tag="velB")
                nc.sync.dma_start(
                    out=velb_t,
                    in_=velt_h[t0 : t0 + lbv, :].rearrange("t b -> (t b)").unsqueeze(0),
                )
                return velb_t

            def emit_readout(k, ytile):
                pro = pp.tile([1, rosz], f32, tag="ro", bufs=1, name="psum_ro")
                for c in range(CB):
                    nc.tensor.matmul(
                        out=pro,
                        lhsT=wrt[:, c : c + 1],
                        rhs=ytile[:, c, :, :].rearrange("p t b -> p (t b)"),
                        start=(c == 0),
                        stop=(c == CB - 1),
                        skip_group_check=True,
                    )
                osb = osbp.tile([1, rosz], f32, tag="osb", name="out_sb")
                nc.vector.tensor_copy(osb, pro)
                nc.sync.dma_start(
                    out=out_h[0:1, k * rosz : (k + 1) * rosz], in_=osb
                )

            # prologue: vel block 0 + its bv outer products
            velb = dma_velb(0)
            bvp_cur = bvpp.tile([128, lbv, CB, B_LOC], f16, tag="bvp")
            for r in range(16):
                build_bv_pair(r, velb, bvp_cur)
            bvp_next = None
            velb_next = None

            yb_cur = None
            yb_prev = None
            for t in range(T):
                blk, j = divmod(t, lbv)
                rob, rj = divmod(t, ro)

                if rj == 0:
                    yb_prev = yb_cur
                    yb_cur = ybp.tile([128, CB, ro, B_LOC], f16, tag="yb")

                if j == 0 and blk + 1 < nblk:
                    velb_next = dma_velb(t + lbv)
                    bvp_next = bvpp.tile([128, lbv, CB, B_LOC], f16, tag="bvp")

                # spread next block's bv build: one pair per 8 steps
                if blk + 1 < nblk and j >= 8 and j % 8 == 0:
                    build_bv_pair(j // 8 - 1, velb_next, bvp_next)
                    if j == 120:
                        build_bv_pair(15, velb_next, bvp_next)

                # batched readout of the previous 32-step block
                if rj == 4 and rob >= 1:
                    emit_readout(rob - 1, yb_prev)

                # ---- the step ----
                # decay + input; block 0 last so its DVE tick covers the rest
                for b in (1, 2, 3, 0):
                    nc.vector.scalar_tensor_tensor(
                        out=psum_z[b],
                        in0=psum_z[b],
                        scalar=float(DECAY),
                        in1=bvp_cur[:, j, b, :],
                        op0=mybir.AluOpType.mult,
                        op1=mybir.AluOpType.add,
                    )
                # recurrent matmuls; first MM (b0,ci3) waits the newest ACT
                # tick of step t-1, covering every other y read this step
                if t == 0:
                    ysl = lambda ci: y0[:, ci, :]
                elif rj == 0:
                    ysl = lambda ci: yb_prev[:, ci, ro - 1, :]
                else:
                    ysl = lambda ci, _s=rj - 1: yb_cur[:, ci, _s, :]
                for b in range(CB):
                    cis = (3, 0, 1, 2) if b == 0 else (0, 1, 2, 3)
                    for ci in cis:
                        nc.tensor.matmul(
                            out=psum_z[b],
                            lhsT=jt[:, ci, 128 * b : 128 * (b + 1)],
                            rhs=ysl(ci),
                            start=False,
                            stop=False,
                            skip_group_check=True,
                        )
                # y_t = tanh(H/64)
                for b in range(CB):
                    nc.scalar.activation(
                        out=yb_cur[:, b, rj, :],
                        in_=psum_z[b],
                        func=mybir.ActivationFunctionType.Tanh,
                        scale=1.0 / HSCALE,
                    )

                if j == lbv - 1 and blk + 1 < nblk:
                    bvp_cur = bvp_next
                    velb = velb_next

            emit_readout(nro - 1, yb_cur)

    nc.compile()
    return nc


_NC_CACHE = {}


def _get_nc(**kw):
    key = tuple(sorted(kw.items()))
    if key not in _NC_CACHE:
        _NC_CACHE[key] = build_nc(**kw)
    return _NC_CACHE[key]


def make_in_maps(vel, J, Bmat, W_ro):
    vel = np.asarray(vel, dtype=np.float32)[:, :, 0]          # [B, T]
    J = np.asarray(J, dtype=np.float32)
    Bmat = np.asarray(Bmat, dtype=np.float32)
    W_ro = np.asarray(W_ro, dtype=np.float32)

    jt = np.ascontiguousarray((HSCALE * DT * J).T).astype(np.float16)
    bmr = np.ascontiguousarray(
        (HSCALE * DT * Bmat[:, 0]).reshape(1, HIDDEN)
    ).astype(np.float16)
    wrt = np.ascontiguousarray(W_ro[0].reshape(CB, 128).T).astype(np.float16)
    return [
        {
            "JT": jt,
            "bmr": bmr,
            "wrt": wrt,
            "velT": np.ascontiguousarray(
                vel[c * B_LOC : (c + 1) * B_LOC].T
            ).astype(np.float16),
        }
        for c in range(N_CORES)
    ]


def kernel(vel, J, Bmat, W_ro, _trace=False, **build_kw):
    from concourse.bass_utils import run_bass_kernel_spmd

    nc = _get_nc(**build_kw)
    in_maps = make_in_maps(vel, J, Bmat, W_ro)
    res = run_bass_kernel_spmd(nc, in_maps, list(range(N_CORES)), trace=_trace)
    # out[0, t*B_LOC + b] = readout(batch row b, step t)
    out = np.stack(
        [r["out"].reshape(T_FULL, B_LOC).T for r in res.results], axis=0
    ).reshape(BATCH, T_FULL)
    out = out[:, :, None].astype(np.float32)
    if _trace:
        kernel.last_results = res
    return out


kernel.last_results = None
